# revision 24
# baseline (speedup 1.0000x reference)
"""Distributed GCN encoder for TRN2: host prep + Bass/Tile kernel builder.

Math (per reference):
  dis = 1/sqrt(deg)   deg = in-degree incl self-loop
  A = D^-1/2 (Adj + I) D^-1/2
  h  = LeakyReLU(A (x W_shared) + b_shared, 0.01)
  mu = A (h W_mu) + b_mu ;  lv = min(A (h W_lv) + b_lv, 10)

Device formulation (per core, nodes sharded in NCORES contiguous ranges):
  t~ = dis * (x @ Wsh)          -> AllGather (bf16)
  seg1_i = sum_{e: dst=i} t~[src_e]    (dma_gather + one-hot matmul segsum)
  z = dis * (seg1 + t~_i)  [+ b] ; h = max(z, .01 z) ; hhat = dis*h
  m2 = hhat @ [W_mu|W_lv]       -> AllGather (bf16)
  seg2 likewise; out = dis * (seg2 + m2_i) [+ b2]; mu = out[:,:64],
  lv = min(out[:,64:], 10)

The scatter-add is a TensorEngine segment-sum: edges are gathered 128 at a
time (dma_gather, int16 indices relative to a 25600-row group window); a
one-hot S[e, m] = (dstslot[e] == m) built on DVE maps each chunk onto the
128 dst rows of its tile; PSUM accumulates all chunks of a tile plus an
identity matmul adding the self-loop term. Dst tiles are processed in
supertile batches of BT so at most BT accumulators live in PSUM.
"""

from contextlib import ExitStack

import numpy as np
import ml_dtypes

BF16 = ml_dtypes.bfloat16
FP8 = ml_dtypes.float8_e4m3fn
P = 128
PAD_SLOT = 200.0


# ----------------------------------------------------------------- config ---
def make_cfg(N, ncores=8, bt=5, call_chunks=48, ngroups=4,
             dma_scratch=16384, nqueues=1, single_packet=True):
    NP = N // ncores
    assert NP * ncores == N
    NT_real = (NP + P - 1) // P
    NT = ((NT_real + bt - 1) // bt) * bt        # pad tile count to mult of BT
    NG = 4                              # pieces (split AllGather blocks)
    H = NP // NG                        # local rows per piece
    assert H * NG == NP
    group = ncores * H                  # table rows per piece block
    assert group <= 32767               # int16 relative index
    return dict(
        N=N, NCORES=ncores, NP=NP, NT=NT, NT_real=NT_real, BT=bt,
        NB=NT // bt, GROUP=group, NG=NG, H=H,
        CALL_CHUNKS=call_chunks, F1=256, F2=128, LAT=64,
        DMA_SCRATCH=dma_scratch, NQUEUES=nqueues,
        SINGLE_PACKET=single_packet,
    )


# ------------------------------------------------------------- host prep ----
def preprocess(cfg, edge_index):
    N, NCORES, NP, NT, BT, NB = (cfg[k] for k in
                                 ("N", "NCORES", "NP", "NT", "BT", "NB"))
    NT_real = cfg["NT_real"]
    GROUP, NG, CALL_CHUNKS = cfg["GROUP"], cfg["NG"], cfg["CALL_CHUNKS"]

    src = np.asarray(edge_index[0], dtype=np.int64)
    dst = np.asarray(edge_index[1], dtype=np.int64)

    deg = np.bincount(dst, minlength=N).astype(np.float32) + 1.0

    H = cfg["H"]
    core = dst // NP
    dloc = dst % NP
    t = dloc // P
    slot = dloc % P
    tb = t // BT
    # piece-block table order: piece g holds rows (c, rloc%H) c-major so the
    # AllGather of local rows [g*H,(g+1)*H) lands contiguously
    c_src = src // NP
    rloc = src % NP
    g = rloc // H
    src_rel = (c_src * H + (rloc % H)).astype(np.int16)
    row = src_rel.astype(np.int64)      # gather-address sort key within piece

    # row as minor key: ascending gather addresses within each (t, g) run
    order = np.lexsort((row, t, g, tb, core))
    src_rel = src_rel[order]
    slot_s = slot[order].astype(np.int32)
    key_core = core[order]
    key_t = t[order]
    key_g = g[order]

    lin = (key_core * NT + key_t) * NG + key_g
    cnt = np.bincount(lin, minlength=NCORES * NT * NG).reshape(NCORES, NT, NG)
    nch = (-(-cnt // P)).max(axis=0)             # [NT, NG] ceil, shared
    nch[:NT_real] = np.maximum(nch[:NT_real], 1)

    seg_chunks = np.zeros((NB, NG, BT), dtype=np.int64)
    for b in range(NB):
        for gg in range(NG):
            for ti in range(BT):
                seg_chunks[b, gg, ti] = nch[b * BT + ti, gg]
    chunk_off = np.concatenate([[0], np.cumsum(seg_chunks.reshape(-1))])
    totch = int(chunk_off[-1])
    tot_slots = totch * P

    def seg_idx(tt, gg):
        b, ti = tt // BT, tt % BT
        return (b * NG + gg) * BT + ti

    idx16 = np.zeros((NCORES, tot_slots), dtype=np.int16)
    slots = np.full((NCORES, tot_slots), PAD_SLOT, dtype=np.float32)

    cnt_stream = np.zeros((NCORES, NB, NG, BT), dtype=np.int64)
    for c in range(NCORES):
        for b in range(NB):
            for gg in range(NG):
                for ti in range(BT):
                    cnt_stream[c, b, gg, ti] = cnt[c, b * BT + ti, gg]
    e_off = np.concatenate([[0], np.cumsum(cnt_stream.reshape(-1))])
    for c in range(NCORES):
        for b in range(NB):
            for gg in range(NG):
                for ti in range(BT):
                    k = ((c * NB + b) * NG + gg) * BT + ti
                    n_e = int(e_off[k + 1] - e_off[k])
                    if n_e == 0:
                        continue
                    tt = b * BT + ti
                    s0 = int(chunk_off[seg_idx(tt, gg)]) * P
                    idx16[c, s0:s0 + n_e] = src_rel[e_off[k]:e_off[k + 1]]
                    slots[c, s0:s0 + n_e] = slot_s[e_off[k]:e_off[k + 1]]

    chunk_tile = np.zeros(totch, dtype=np.int32)
    pos = 0
    for b in range(NB):
        for gg in range(NG):
            for ti in range(BT):
                n_ = int(seg_chunks[b, gg, ti])
                chunk_tile[pos:pos + n_] = b * BT + ti
                pos += n_

    calls = []
    pos = 0
    for b in range(NB):
        for gg in range(NG):
            seg = int(seg_chunks[b, gg].sum())
            o = 0
            while o < seg:
                n_ = min(CALL_CHUNKS, seg - o)
                calls.append((pos + o, pos + o + n_, gg))
                o += n_
            pos += seg

    cols = tot_slots // 16
    idx_w = np.zeros((NCORES, P, cols), dtype=np.int16)
    slot_w = np.zeros((NCORES, P, totch), dtype=BF16)
    for c in range(NCORES):
        idx_w[c] = np.tile(idx16[c].reshape(cols, 16).T, (8, 1))
        slot_w[c] = slots[c].reshape(totch, P).T.astype(BF16)

    deg_w = np.ones((NCORES, P, NT), dtype=np.float32)
    for c in range(NCORES):
        d = deg[c * NP:(c + 1) * NP]
        pad = np.ones(NT * P - NP, dtype=np.float32)
        deg_w[c] = np.concatenate([d, pad]).reshape(NT, P).T

    first_ch = np.full(NT, -1, dtype=np.int64)
    last_ch = np.full(NT, -1, dtype=np.int64)
    for ch in range(totch):
        tt = int(chunk_tile[ch])
        if first_ch[tt] < 0:
            first_ch[tt] = ch
        last_ch[tt] = ch

    # chunk index at the start of each supertile batch (metadata slab bounds)
    batch_bounds = [int(chunk_off[b * NG * BT]) for b in range(NB)] + [totch]

    return dict(
        deg=deg, idx_w=idx_w, slot_w=slot_w, deg_w=deg_w,
        chunk_tile=chunk_tile, calls=calls, totch=totch,
        first_ch=first_ch, last_ch=last_ch, batch_bounds=batch_bounds,
    )


def stage_host(cfg, pre, x, W_shared, b_shared, W_mu, b_mu, W_lv, b_lv):
    """Build per-core in_maps (list of dicts of numpy arrays)."""
    NCORES, NP, NT, F2 = (cfg[k] for k in ("NCORES", "NP", "NT", "F2"))
    NPP = NT * P
    xT = np.zeros((NCORES, P, 2 * NPP), dtype=BF16)
    for c in range(NCORES):
        xl = np.zeros((NPP, 256), dtype=BF16)
        xl[:NP] = np.asarray(x[c * NP:(c + 1) * NP]).astype(BF16)
        for a in range(2):
            xT[c, :, a * NPP:(a + 1) * NPP] = xl[:, a * P:(a + 1) * P].T
    Wsh = np.zeros((P, 2 * 256), dtype=BF16)
    for a in range(2):
        Wsh[:, a * 256:(a + 1) * 256] = W_shared[a * P:(a + 1) * P, :].astype(BF16)
    W2f = np.concatenate([W_mu, W_lv], axis=1)
    W2 = np.zeros((P, 2 * F2), dtype=BF16)
    for a in range(2):
        W2[:, a * F2:(a + 1) * F2] = W2f[a * P:(a + 1) * P, :].astype(BF16)
    iota = np.ascontiguousarray(
        np.broadcast_to(np.arange(P, dtype=np.float32), (P, P))).astype(BF16)
    ident = np.eye(P, dtype=np.float32).astype(BF16)
    ident8 = np.eye(P, dtype=np.float32).astype(FP8)

    has_b1 = bool(np.any(np.asarray(b_shared) != 0))
    has_b2 = bool(np.any(np.asarray(b_mu) != 0) or np.any(np.asarray(b_lv) != 0))
    brep = np.broadcast_to(np.asarray(b_shared, np.float32), (P, 256)).copy()
    b2rep = np.broadcast_to(
        np.concatenate([np.asarray(b_mu), np.asarray(b_lv)]).astype(np.float32),
        (P, 2 * cfg["LAT"])).copy()

    in_maps = []
    for c in range(NCORES):
        m = dict(
            xT=xT[c], Wsh=Wsh, W2=W2, iota=iota, ident=ident, ident8=ident8,
            degw=pre["deg_w"][c], idxw=pre["idx_w"][c], slotw=pre["slot_w"][c],
        )
        if has_b1:
            m["brep"] = brep
        if has_b2:
            m["b2rep"] = b2rep
        in_maps.append(m)
    return in_maps, has_b1, has_b2


# -------------------------------------------------------------- kernel ------
def build_kernel(cfg, pre, has_b1, has_b2, compat=True):
    import concourse.bass as bass
    import concourse.bacc as bacc
    import concourse.mybir as mybir
    import concourse.tile as tile

    N, NCORES, NP, NT, BT, NB = (cfg[k] for k in
                                 ("N", "NCORES", "NP", "NT", "BT", "NB"))
    GROUP, NG = cfg["GROUP"], cfg["NG"]
    F1, F2, LAT = cfg["F1"], cfg["F2"], cfg["LAT"]
    NPP = NT * P
    totch = pre["totch"]
    calls = pre["calls"]
    chunk_tile = pre["chunk_tile"]
    first_ch, last_ch = pre["first_ch"], pre["last_ch"]
    bf = mybir.dt.bfloat16
    f32 = mybir.dt.float32
    bf8 = mybir.dt.float8e4
    use_fp8 = cfg.get('FP8', True)
    dt1 = bf8 if use_fp8 else bf          # L1 gather-table dtype

    nc = bacc.Bacc(num_devices=NCORES,
                   dynamic_dma_scratch_size=cfg.get('DMA_SCRATCH', 16384),
                   num_swdge_queues=cfg.get('NQUEUES', 1))
    xT = nc.declare_dram_parameter("xT", [P, 2 * NPP], bf, isOutput=False)
    Wsh = nc.declare_dram_parameter("Wsh", [P, 2 * 256], bf, isOutput=False)
    W2 = nc.declare_dram_parameter("W2", [P, 2 * F2], bf, isOutput=False)
    iota = nc.declare_dram_parameter("iota", [P, P], bf, isOutput=False)
    ident = nc.declare_dram_parameter("ident", [P, P], bf, isOutput=False)
    ident8 = nc.declare_dram_parameter("ident8", [P, P], bf8, isOutput=False)
    degw = nc.declare_dram_parameter("degw", [P, NT], f32, isOutput=False)
    idxw = nc.declare_dram_parameter("idxw", [P, totch * 8], mybir.dt.int16,
                                     isOutput=False)
    slotw = nc.declare_dram_parameter("slotw", [P, totch], bf, isOutput=False)
    brep = b2rep = None
    if has_b1:
        brep = nc.declare_dram_parameter("brep", [P, 256], f32, isOutput=False)
    if has_b2:
        b2rep = nc.declare_dram_parameter("b2rep", [P, 2 * LAT], f32,
                                          isOutput=False)
    mu_out = nc.declare_dram_parameter("mu", [NP, LAT], f32, isOutput=True)
    lv_out = nc.declare_dram_parameter("lv", [NP, LAT], f32, isOutput=True)

    H = cfg["H"]
    cc_in1_p = [nc.dram_tensor(f"cc_in1_{p}", [H, F1], dt1) for p in range(NG)]
    cc_out1_p = [nc.dram_tensor(f"cc_out1_{p}", [NCORES * H, F1], dt1,
                                addr_space="Shared") for p in range(NG)]
    cc_in2_p = [nc.dram_tensor(f"cc_in2_{p}", [H, F2], bf) for p in range(NG)]
    cc_out2_p = [nc.dram_tensor(f"cc_out2_{p}", [NCORES * H, F2], bf,
                                addr_space="Shared") for p in range(NG)]

    groups = [list(range(NCORES))]

    def rows_of(t):
        return max(0, min(P, NP - t * P))

    with tile.TileContext(nc) as tc, ExitStack() as ctx:
        mm = lambda *a, **k: nc.tensor.matmul(*a, skip_group_check=True, **k)

        cp = ctx.enter_context(tc.tile_pool(name="const", bufs=1))
        iota_sb = cp.tile([P, P], bf)
        ident_sb = cp.tile([P, P], bf)
        ident8_sb = cp.tile([P, P], bf8)
        nc.sync.dma_start(out=ident8_sb[:], in_=ident8[:])
        Wsh_sb = cp.tile([P, 2 * 256], bf)
        W2_sb = cp.tile([P, 2 * F2], bf)
        deg_sb = cp.tile([P, NT], f32)
        dis_sb = cp.tile([P, NT], f32)
        nc.sync.dma_start(out=iota_sb[:], in_=iota[:])
        nc.sync.dma_start(out=ident_sb[:], in_=ident[:])
        nc.sync.dma_start(out=Wsh_sb[:], in_=Wsh[:])
        nc.sync.dma_start(out=W2_sb[:], in_=W2[:])
        nc.sync.dma_start(out=deg_sb[:], in_=degw[:])
        nc.vector.reciprocal(dis_sb[:], deg_sb[:])
        nc.scalar.activation(dis_sb[:], dis_sb[:],
                             mybir.ActivationFunctionType.Sqrt)
        brep_sb = b2rep_sb = None
        if has_b1:
            brep_sb = cp.tile([P, 256], f32)
            nc.sync.dma_start(out=brep_sb[:], in_=brep[:])
        if has_b2:
            b2rep_sb = cp.tile([P, 2 * LAT], f32)
            nc.sync.dma_start(out=b2rep_sb[:], in_=b2rep[:])

        tloc = cp.tile([P, NT * F1], dt1)    # t~ local rows (tile-major)
        m2loc = cp.tile([P, NT * F2], bf)    # m2~ local rows


        # one gpsimd register per distinct gather length (reused across calls)
        nidx_reg = {}
        for (c0, c1, _g) in calls:
            n_ = (c1 - c0) * P
            if n_ not in nidx_reg:
                nidx_reg[n_] = nc.gpsimd.to_reg(n_)

        def piece_write(dests, r0, hi, sb_rows):
            """DMA sb_rows[:hi] to local rows [r0, r0+hi), split at piece
            boundaries so each piece tensor's deps are exact."""
            a = r0
            while a < r0 + hi:
                p = a // H
                b = min(r0 + hi, (p + 1) * H)
                nc.sync.dma_start(out=dests[p][a - p * H: b - p * H, :],
                                  in_=sb_rows[a - r0: b - r0, :])
                a = b

        # ---------------- phase 0: transform + t~ -> cc_in1 -----------------
        TB0 = 4                                  # x tiles loaded per DMA
        with tc.tile_pool(name="ph0", bufs=3) as p0, \
             tc.tile_pool(name="ph0ps", bufs=4, space="PSUM") as p0ps:
            for t0 in range(0, NT, TB0):
                ntl = min(TB0, NT - t0)
                xa = []
                for a in range(2):
                    xt4 = p0.tile([P, ntl * P], bf, tag=f"xa{a}")
                    nc.sync.dma_start(
                        out=xt4[:],
                        in_=xT[:, a * NPP + t0 * P: a * NPP + (t0 + ntl) * P])
                    xa.append(xt4)
                for ti in range(ntl):
                    t = t0 + ti
                    hi = rows_of(t)
                    if hi == 0:
                        continue
                    ps = p0ps.tile([P, F1], f32, tag="tps")
                    for a in range(2):
                        mm(ps[:], lhsT=xa[a][:, ti * P:(ti + 1) * P],
                           rhs=Wsh_sb[:, a * 256:(a + 1) * 256],
                           start=(a == 0), stop=(a == 1))
                    dst = tloc[:, t * F1:(t + 1) * F1]
                    nc.scalar.activation(dst, ps[:],
                                         mybir.ActivationFunctionType.Copy,
                                         scale=dis_sb[:, t:t + 1])
                    piece_write(cc_in1_p, t * P, hi, dst)

        def issue_ag(cc_in, cc_out):
            nc.gpsimd.collective_compute(
                "AllGather", mybir.AluOpType.bypass, replica_groups=groups,
                ins=[cc_in[:, :]], outs=[cc_out[:, :]])

        for p in range(NG):
            issue_ag(cc_in1_p[p], cc_out1_p[p])

        # ---------------- propagate (shared L1/L2) --------------------------
        batch_bounds = pre["batch_bounds"]
        calls_by_batch = [[] for _ in range(NB)]
        for (c0, c1, gg) in calls:
            for b in range(NB):
                if batch_bounds[b] <= c0 and c1 <= batch_bounds[b + 1]:
                    calls_by_batch[b].append((c0, c1, gg))
                    break
            else:
                raise AssertionError("call crosses batch bound")

        gbufs = cfg.get('GBUFS', 8)
        gp = ctx.enter_context(tc.tile_pool(name="gat", bufs=gbufs))
        mp = ctx.enter_context(tc.tile_pool(name="meta", bufs=3))
        sp = ctx.enter_context(tc.tile_pool(name="s", bufs=gbufs))
        ep = ctx.enter_context(tc.tile_pool(name="eps", bufs=4))
        qctr = [0]                      # shared gather-queue rotation

        def propagate(layer, tables, F, self_tiles, epilogue, acc_bufs, pa,
                      dt, ident_self, post_batch=None, cfg=cfg):
            psum_of = {}
            for b in range(NB):
                bc0, bc1 = batch_bounds[b], batch_bounds[b + 1]
                nbc = bc1 - bc0
                if nbc == 0:
                    continue
                # one metadata slab DMA pair per supertile batch
                idx_sl = mp.tile([P, nbc * 8], mybir.dt.int16,
                                 tag=f"idx{layer}")
                nc.sync.dma_start(out=idx_sl[:],
                                  in_=idxw[:, bc0 * 8: bc1 * 8])
                slot_sl = mp.tile([P, nbc], bf, tag=f"slot{layer}")
                nc.sync.dma_start(out=slot_sl[:], in_=slotw[:, bc0:bc1])
                for (c0, c1, gg) in calls_by_batch[b]:
                    nch_call = c1 - c0
                    nidx = nch_call * P
                    idx_t = idx_sl[:, (c0 - bc0) * 8:(c1 - bc0) * 8]
                    S_t = sp.tile([P, nch_call * P], dt, tag=f"S{layer}")
                    iota_b = bass.AP(iota_sb[:].tensor, iota_sb[:].offset,
                                     [list(iota_sb[:].ap[0]), [0, nch_call],
                                      list(iota_sb[:].ap[1])])
                    sbase = slot_sl[:, c0 - bc0: c1 - bc0]
                    slot_b = bass.AP(sbase.tensor, sbase.offset,
                                     [list(sbase.ap[0]),
                                      list(sbase.ap[1]), [0, P]])
                    nc.vector.tensor_tensor(out=S_t[:], in0=iota_b,
                                            in1=slot_b,
                                            op=mybir.AluOpType.is_equal)
                    gath = gp.tile([P, nch_call, F], dt, tag=f"g{layer}")
                    q = qctr[0] % cfg.get('NQUEUES', 1)
                    qctr[0] += 1
                    nc.gpsimd.dma_gather(
                        gath[:], tables[gg][:, :],
                        idx_t, nidx, nidx_reg[nidx], F,
                        queue_num=q,
                        single_packet=cfg.get('SINGLE_PACKET', True),
                    )
                    for ch in range(c0, c1):
                        t = int(chunk_tile[ch])
                        if ch == first_ch[t]:
                            ps = pa.tile([P, F], f32, tag=f"acc{layer}")
                            psum_of[t] = ps
                            mm(ps[:], lhsT=ident_self[:],
                               rhs=self_tiles[:, t * F:(t + 1) * F],
                               start=True, stop=False)
                        ps = psum_of[t]
                        mm(ps[:],
                           lhsT=S_t[:, (ch - c0) * P:(ch - c0 + 1) * P],
                           rhs=gath[:, ch - c0, :],
                           start=False, stop=(ch == last_ch[t]))
                        if ch == last_ch[t]:
                            epilogue(t, ps, ep)
                            del psum_of[t]
                if post_batch is not None:
                    post_batch(b)

        # ---------------- L1 epilogue: h, hhat, m2 --------------------------
        l1_ctx = ExitStack()
        pa1 = l1_ctx.enter_context(tc.tile_pool(name="acc1", bufs=BT,
                                                space="PSUM"))
        tp_pool = l1_ctx.enter_context(tc.tile_pool(name="tp", bufs=2,
                                                    space="PSUM"))
        m2_pool = l1_ctx.enter_context(tc.tile_pool(name="m2", bufs=1,
                                                    space="PSUM"))

        def epi1(t, ps, ep):
            hi = rows_of(t)
            if has_b1:
                z = ep.tile([P, F1], f32, tag="z")
                nc.scalar.activation(z[:], ps[:],
                                     mybir.ActivationFunctionType.Copy,
                                     scale=dis_sb[:, t:t + 1])
                nc.vector.tensor_tensor(out=z[:], in0=z[:], in1=brep_sb[:],
                                        op=mybir.AluOpType.add)
                h = ep.tile([P, F1], f32, tag="h")
                nc.scalar.activation(h[:], z[:],
                                     mybir.ActivationFunctionType.Lrelu,
                                     alpha=0.01)
            else:
                # h = LeakyReLU(dis * seg, 0.01) fused on the Scalar engine
                h = ep.tile([P, F1], f32, tag="h")
                nc.scalar.activation(h[:], ps[:],
                                     mybir.ActivationFunctionType.Lrelu,
                                     scale=dis_sb[:, t:t + 1], alpha=0.01)
            hhat = ep.tile([P, F1], bf, tag="hh")
            nc.scalar.activation(hhat[:], h[:],
                                 mybir.ActivationFunctionType.Copy,
                                 scale=dis_sb[:, t:t + 1])
            m2ps = m2_pool.tile([P, F2], f32, tag="m2ps")
            for a in range(2):
                tp = tp_pool.tile([P, P], bf, tag="tp")
                nc.tensor.transpose(tp[:], hhat[:, a * P:(a + 1) * P],
                                    ident_sb[:])
                hT = ep.tile([P, P], bf, tag="hT")
                nc.scalar.activation(hT[:], tp[:],
                                     mybir.ActivationFunctionType.Copy)
                mm(m2ps[:], lhsT=hT[:],
                   rhs=W2_sb[:, a * F2:(a + 1) * F2],
                   start=(a == 0), stop=(a == 1))
            dst = m2loc[:, t * F2:(t + 1) * F2]
            nc.scalar.activation(dst, m2ps[:],
                                 mybir.ActivationFunctionType.Copy)
            piece_write(cc_in2_p, t * P, hi, dst)

        # AG2 piece p fires once the last tile of piece p has its epilogue
        # issued (plus pipeline margin), overlapping the rest of prop1
        ag2_at = {}
        for p in range(NG):
            last_tile = -(-((p + 1) * H) // P) - 1
            bb = min(last_tile // BT + 2, NB - 1)
            ag2_at.setdefault(bb, []).append(p)
        ag2_done = set()

        def post_batch1(b):
            for p in ag2_at.get(b, []):
                if p not in ag2_done:
                    ag2_done.add(p)
                    issue_ag(cc_in2_p[p], cc_out2_p[p])

        propagate(1, cc_out1_p, F1, tloc, epi1, acc_bufs=BT, pa=pa1,
                  dt=dt1, ident_self=ident8_sb if use_fp8 else ident_sb,
                  post_batch=post_batch1)
        for p in range(NG):
            if p not in ag2_done:
                issue_ag(cc_in2_p[p], cc_out2_p[p])
        l1_ctx.close()

        # ---------------- L2 epilogue: mu / lv ------------------------------
        def epi2(t, ps, ep):
            hi = rows_of(t)
            muv = ep.tile([P, LAT], f32, tag="mu")
            lvv = ep.tile([P, LAT], f32, tag="lv")
            if has_b2:
                o2 = ep.tile([P, 2 * LAT], f32, tag="o2")
                nc.vector.tensor_scalar_mul(o2[:], ps[:], dis_sb[:, t:t + 1])
                nc.vector.tensor_tensor(out=o2[:], in0=o2[:], in1=b2rep_sb[:],
                                        op=mybir.AluOpType.add)
                nc.vector.tensor_copy(out=muv[:], in_=o2[:, :LAT])
                nc.vector.tensor_scalar(out=lvv[:], in0=o2[:, LAT:],
                                        scalar1=10.0, scalar2=None,
                                        op0=mybir.AluOpType.min)
            else:
                nc.scalar.activation(muv[:], ps[:, :LAT],
                                     mybir.ActivationFunctionType.Copy,
                                     scale=dis_sb[:, t:t + 1])
                nc.vector.tensor_scalar(out=lvv[:], in0=ps[:, LAT:],
                                        scalar1=dis_sb[:, t:t + 1],
                                        scalar2=10.0,
                                        op0=mybir.AluOpType.mult,
                                        op1=mybir.AluOpType.min)
            nc.sync.dma_start(out=mu_out[t * P: t * P + hi, :],
                              in_=muv[:hi, :])
            nc.sync.dma_start(out=lv_out[t * P: t * P + hi, :],
                              in_=lvv[:hi, :])

        with tc.tile_pool(name="acc2", bufs=BT + 2, space="PSUM") as pa2:
            propagate(2, cc_out2_p, F2, m2loc, epi2, acc_bufs=BT + 2, pa=pa2,
                      dt=bf, ident_self=ident_sb)

    return nc


# ======================================================================
# Public entry point
# ======================================================================
def kernel(**inputs):
    """Full-input distributed GCN encoder on 8 TRN2 NeuronCores.

    Takes the unsharded inputs of reference.setup_inputs(), shards nodes
    across the 8 cores, runs the Bass kernel via run_bass_kernel_spmd,
    and returns the full (mu, logvar) tuple.
    """
    import os
    import sys
    import types

    x = np.asarray(inputs["x"], dtype=np.float32)
    edge_index = np.asarray(inputs["edge_index"])
    W_shared = np.asarray(inputs["W_shared"], dtype=np.float32)
    b_shared = np.asarray(inputs["b_shared"], dtype=np.float32)
    W_mu = np.asarray(inputs["W_mu"], dtype=np.float32)
    b_mu = np.asarray(inputs["b_mu"], dtype=np.float32)
    W_lv = np.asarray(inputs["W_lv"], dtype=np.float32)
    b_lv = np.asarray(inputs["b_lv"], dtype=np.float32)

    N = x.shape[0]
    cfg = make_cfg(
        N, ncores=8,
        call_chunks=int(os.environ.get("GCN_CALL_CHUNKS", "8")),
        nqueues=int(os.environ.get("GCN_NQUEUES", "4")),
        bt=int(os.environ.get("GCN_BT", "5")),
        dma_scratch=int(os.environ.get("GCN_DMA_SCRATCH", "32768")),
    )
    cfg['GBUFS'] = int(os.environ.get("GCN_GBUFS", "10"))
    cfg['SINGLE_PACKET'] = os.environ.get("GCN_SP", "1") == "1"
    cfg['FP8'] = os.environ.get("GCN_FP8", "0") == "1"
    cfg['AGX'] = int(os.environ.get("GCN_AGX", "1"))
    pre = preprocess(cfg, edge_index)
    in_maps, has_b1, has_b2 = stage_host(
        cfg, pre, x, W_shared, b_shared, W_mu, b_mu, W_lv, b_lv)
    nc = build_kernel(cfg, pre, has_b1, has_b2)
    nc.finalize()

    from concourse.bass_utils import run_bass_kernel_spmd

    trace = bool(int(os.environ.get("GCN_KERNEL_TRACE", "0")))
    if trace:
        # register the NTFF profiling hook this container ships without
        try:
            import trn_agent_boot.trn_boot as _tb
            _hook = _tb._ntff_profile_via_ctypes("/opt/axon/libaxon_pjrt.so")
            _m = types.ModuleType("antenv.axon_hooks")
            _m.get_axon_ntff_profile_hook = lambda: _hook
            sys.modules["antenv.axon_hooks"] = _m
        except Exception:
            trace = False

    res = run_bass_kernel_spmd(nc, in_maps, core_ids=list(range(cfg["NCORES"])),
                               trace=trace)
    kernel.last_exec_time_ns = res.exec_time_ns
    mu = np.concatenate([res.results[c]["mu"] for c in range(cfg["NCORES"])])
    lv = np.concatenate([res.results[c]["lv"] for c in range(cfg["NCORES"])])
    return mu.astype(np.float32), lv.astype(np.float32)


kernel.last_exec_time_ns = None



# revision 30
# speedup vs baseline: 1.1351x; 1.1351x over previous
"""Distributed GCN encoder for TRN2: host prep + Bass/Tile kernel builder.

Math (per reference):
  dis = 1/sqrt(deg)   deg = in-degree incl self-loop
  A = D^-1/2 (Adj + I) D^-1/2
  h  = LeakyReLU(A (x W_shared) + b_shared, 0.01)
  mu = A (h W_mu) + b_mu ;  lv = min(A (h W_lv) + b_lv, 10)

Device formulation (per core, nodes sharded in NCORES contiguous ranges):
  t~ = dis * (x @ Wsh)          -> AllGather (bf16)
  seg1_i = sum_{e: dst=i} t~[src_e]    (dma_gather + one-hot matmul segsum)
  z = dis * (seg1 + t~_i)  [+ b] ; h = max(z, .01 z) ; hhat = dis*h
  m2 = hhat @ [W_mu|W_lv]       -> AllGather (bf16)
  seg2 likewise; out = dis * (seg2 + m2_i) [+ b2]; mu = out[:,:64],
  lv = min(out[:,64:], 10)

The scatter-add is a TensorEngine segment-sum: edges are gathered 128 at a
time (dma_gather, int16 indices relative to a 25600-row group window); a
one-hot S[e, m] = (dstslot[e] == m) built on DVE maps each chunk onto the
128 dst rows of its tile; PSUM accumulates all chunks of a tile plus an
identity matmul adding the self-loop term. Dst tiles are processed in
supertile batches of BT so at most BT accumulators live in PSUM.
"""

from contextlib import ExitStack

import numpy as np
import ml_dtypes

BF16 = ml_dtypes.bfloat16
FP8 = ml_dtypes.float8_e4m3fn
P = 128
PAD_SLOT = 200.0


# ----------------------------------------------------------------- config ---
def make_cfg(N, ncores=8, bt=5, call_chunks=48, ngroups=4,
             dma_scratch=16384, nqueues=1, single_packet=True):
    NP = N // ncores
    assert NP * ncores == N
    NT_real = (NP + P - 1) // P
    NT = ((NT_real + bt - 1) // bt) * bt        # pad tile count to mult of BT
    NG = 4                              # pieces (split AllGather blocks)
    h0 = int((NP * 0.3) // P * P)       # big pieces; small last piece so the
    HS = [h0, h0, h0, NP - 3 * h0]      # final AllGather piece is quick
    assert all(ncores * h <= 32767 for h in HS)   # int16 relative index
    bounds = [0]
    for h in HS:
        bounds.append(bounds[-1] + h)
    return dict(
        N=N, NCORES=ncores, NP=NP, NT=NT, NT_real=NT_real, BT=bt,
        NB=NT // bt, NG=NG, HS=HS, BOUNDS=bounds,
        CALL_CHUNKS=call_chunks, F1=256, F2=128, LAT=64,
        DMA_SCRATCH=dma_scratch, NQUEUES=nqueues,
        SINGLE_PACKET=single_packet,
    )


# ------------------------------------------------------------- host prep ----
def preprocess(cfg, edge_index):
    N, NCORES, NP, NT, BT, NB = (cfg[k] for k in
                                 ("N", "NCORES", "NP", "NT", "BT", "NB"))
    NT_real = cfg["NT_real"]
    NG, CALL_CHUNKS = cfg["NG"], cfg["CALL_CHUNKS"]

    src = np.asarray(edge_index[0], dtype=np.int64)
    dst = np.asarray(edge_index[1], dtype=np.int64)

    deg = np.bincount(dst, minlength=N).astype(np.float32) + 1.0

    bounds = np.asarray(cfg["BOUNDS"], dtype=np.int64)
    hs = np.asarray(cfg["HS"], dtype=np.int64)
    core = dst // NP
    dloc = dst % NP
    t = dloc // P
    slot = dloc % P
    tb = t // BT
    # piece-block table order: piece g holds rows (c, rloc-bounds[g]) c-major
    # so the AllGather of local rows [bounds[g], bounds[g+1]) is contiguous
    c_src = src // NP
    rloc = src % NP
    g = np.searchsorted(bounds, rloc, side="right") - 1
    src_rel = (c_src * hs[g] + (rloc - bounds[g])).astype(np.int16)
    row = src_rel.astype(np.int64)      # gather-address sort key within piece

    # row as minor key: ascending gather addresses within each (t, g) run
    order = np.lexsort((row, t, g, tb, core))
    src_rel = src_rel[order]
    slot_s = slot[order].astype(np.int32)
    key_core = core[order]
    key_t = t[order]
    key_g = g[order]

    lin = (key_core * NT + key_t) * NG + key_g
    cnt = np.bincount(lin, minlength=NCORES * NT * NG).reshape(NCORES, NT, NG)
    nch = (-(-cnt // P)).max(axis=0)             # [NT, NG] ceil, shared
    nch[:NT_real] = np.maximum(nch[:NT_real], 1)

    seg_chunks = np.zeros((NB, NG, BT), dtype=np.int64)
    for b in range(NB):
        for gg in range(NG):
            for ti in range(BT):
                seg_chunks[b, gg, ti] = nch[b * BT + ti, gg]
    chunk_off = np.concatenate([[0], np.cumsum(seg_chunks.reshape(-1))])
    totch = int(chunk_off[-1])
    tot_slots = totch * P

    def seg_idx(tt, gg):
        b, ti = tt // BT, tt % BT
        return (b * NG + gg) * BT + ti

    idx16 = np.zeros((NCORES, tot_slots), dtype=np.int16)
    slots = np.full((NCORES, tot_slots), PAD_SLOT, dtype=np.float32)

    cnt_stream = np.zeros((NCORES, NB, NG, BT), dtype=np.int64)
    for c in range(NCORES):
        for b in range(NB):
            for gg in range(NG):
                for ti in range(BT):
                    cnt_stream[c, b, gg, ti] = cnt[c, b * BT + ti, gg]
    e_off = np.concatenate([[0], np.cumsum(cnt_stream.reshape(-1))])
    for c in range(NCORES):
        for b in range(NB):
            for gg in range(NG):
                for ti in range(BT):
                    k = ((c * NB + b) * NG + gg) * BT + ti
                    n_e = int(e_off[k + 1] - e_off[k])
                    if n_e == 0:
                        continue
                    tt = b * BT + ti
                    s0 = int(chunk_off[seg_idx(tt, gg)]) * P
                    idx16[c, s0:s0 + n_e] = src_rel[e_off[k]:e_off[k + 1]]
                    slots[c, s0:s0 + n_e] = slot_s[e_off[k]:e_off[k + 1]]

    chunk_tile = np.zeros(totch, dtype=np.int32)
    pos = 0
    for b in range(NB):
        for gg in range(NG):
            for ti in range(BT):
                n_ = int(seg_chunks[b, gg, ti])
                chunk_tile[pos:pos + n_] = b * BT + ti
                pos += n_

    calls = []
    pos = 0
    for b in range(NB):
        for gg in range(NG):
            seg = int(seg_chunks[b, gg].sum())
            o = 0
            while o < seg:
                n_ = min(CALL_CHUNKS, seg - o)
                calls.append((pos + o, pos + o + n_, gg))
                o += n_
            pos += seg

    cols = tot_slots // 16
    idx_w = np.zeros((NCORES, P, cols), dtype=np.int16)
    slot_w = np.zeros((NCORES, P, totch), dtype=BF16)
    for c in range(NCORES):
        idx_w[c] = np.tile(idx16[c].reshape(cols, 16).T, (8, 1))
        slot_w[c] = slots[c].reshape(totch, P).T.astype(BF16)

    deg_w = np.ones((NCORES, P, NT), dtype=np.float32)
    for c in range(NCORES):
        d = deg[c * NP:(c + 1) * NP]
        pad = np.ones(NT * P - NP, dtype=np.float32)
        deg_w[c] = np.concatenate([d, pad]).reshape(NT, P).T

    first_ch = np.full(NT, -1, dtype=np.int64)
    last_ch = np.full(NT, -1, dtype=np.int64)
    for ch in range(totch):
        tt = int(chunk_tile[ch])
        if first_ch[tt] < 0:
            first_ch[tt] = ch
        last_ch[tt] = ch

    # chunk index at the start of each supertile batch (metadata slab bounds)
    batch_bounds = [int(chunk_off[b * NG * BT]) for b in range(NB)] + [totch]

    return dict(
        deg=deg, idx_w=idx_w, slot_w=slot_w, deg_w=deg_w,
        chunk_tile=chunk_tile, calls=calls, totch=totch,
        first_ch=first_ch, last_ch=last_ch, batch_bounds=batch_bounds,
    )


def stage_host(cfg, pre, x, W_shared, b_shared, W_mu, b_mu, W_lv, b_lv):
    """Build per-core in_maps (list of dicts of numpy arrays)."""
    NCORES, NP, NT, F2 = (cfg[k] for k in ("NCORES", "NP", "NT", "F2"))
    NPP = NT * P
    xT = np.zeros((NCORES, P, 2 * NPP), dtype=BF16)
    for c in range(NCORES):
        xl = np.zeros((NPP, 256), dtype=BF16)
        xl[:NP] = np.asarray(x[c * NP:(c + 1) * NP]).astype(BF16)
        for a in range(2):
            xT[c, :, a * NPP:(a + 1) * NPP] = xl[:, a * P:(a + 1) * P].T
    Wsh = np.zeros((P, 2 * 256), dtype=BF16)
    for a in range(2):
        Wsh[:, a * 256:(a + 1) * 256] = W_shared[a * P:(a + 1) * P, :].astype(BF16)
    W2f = np.concatenate([W_mu, W_lv], axis=1)
    W2 = np.zeros((P, 2 * F2), dtype=BF16)
    for a in range(2):
        W2[:, a * F2:(a + 1) * F2] = W2f[a * P:(a + 1) * P, :].astype(BF16)
    iota = np.ascontiguousarray(
        np.broadcast_to(np.arange(P, dtype=np.float32), (P, P))).astype(BF16)
    ident = np.eye(P, dtype=np.float32).astype(BF16)
    ident8 = np.eye(P, dtype=np.float32).astype(FP8)

    has_b1 = bool(np.any(np.asarray(b_shared) != 0))
    has_b2 = bool(np.any(np.asarray(b_mu) != 0) or np.any(np.asarray(b_lv) != 0))
    brep = np.broadcast_to(np.asarray(b_shared, np.float32), (P, 256)).copy()
    b2rep = np.broadcast_to(
        np.concatenate([np.asarray(b_mu), np.asarray(b_lv)]).astype(np.float32),
        (P, 2 * cfg["LAT"])).copy()

    in_maps = []
    for c in range(NCORES):
        m = dict(
            xT=xT[c], Wsh=Wsh, W2=W2, iota=iota, ident=ident, ident8=ident8,
            degw=pre["deg_w"][c], idxw=pre["idx_w"][c], slotw=pre["slot_w"][c],
        )
        if has_b1:
            m["brep"] = brep
        if has_b2:
            m["b2rep"] = b2rep
        in_maps.append(m)
    return in_maps, has_b1, has_b2


# -------------------------------------------------------------- kernel ------
def build_kernel(cfg, pre, has_b1, has_b2, compat=True):
    import concourse.bass as bass
    import concourse.bacc as bacc
    import concourse.mybir as mybir
    import concourse.tile as tile

    N, NCORES, NP, NT, BT, NB = (cfg[k] for k in
                                 ("N", "NCORES", "NP", "NT", "BT", "NB"))
    NG = cfg["NG"]
    F1, F2, LAT = cfg["F1"], cfg["F2"], cfg["LAT"]
    NPP = NT * P
    totch = pre["totch"]
    calls = pre["calls"]
    chunk_tile = pre["chunk_tile"]
    first_ch, last_ch = pre["first_ch"], pre["last_ch"]
    bf = mybir.dt.bfloat16
    f32 = mybir.dt.float32
    bf8 = mybir.dt.float8e4
    use_fp8 = cfg.get('FP8', True)
    dt1 = bf8 if use_fp8 else bf          # L1 gather-table dtype

    nc = bacc.Bacc(num_devices=NCORES,
                   dynamic_dma_scratch_size=cfg.get('DMA_SCRATCH', 16384),
                   num_swdge_queues=cfg.get('NQUEUES', 1))
    xT = nc.declare_dram_parameter("xT", [P, 2 * NPP], bf, isOutput=False)
    Wsh = nc.declare_dram_parameter("Wsh", [P, 2 * 256], bf, isOutput=False)
    W2 = nc.declare_dram_parameter("W2", [P, 2 * F2], bf, isOutput=False)
    iota = nc.declare_dram_parameter("iota", [P, P], bf, isOutput=False)
    ident = nc.declare_dram_parameter("ident", [P, P], bf, isOutput=False)
    ident8 = nc.declare_dram_parameter("ident8", [P, P], bf8, isOutput=False)
    degw = nc.declare_dram_parameter("degw", [P, NT], f32, isOutput=False)
    idxw = nc.declare_dram_parameter("idxw", [P, totch * 8], mybir.dt.int16,
                                     isOutput=False)
    slotw = nc.declare_dram_parameter("slotw", [P, totch], bf, isOutput=False)
    brep = b2rep = None
    if has_b1:
        brep = nc.declare_dram_parameter("brep", [P, 256], f32, isOutput=False)
    if has_b2:
        b2rep = nc.declare_dram_parameter("b2rep", [P, 2 * LAT], f32,
                                          isOutput=False)
    mu_out = nc.declare_dram_parameter("mu", [NP, LAT], f32, isOutput=True)
    lv_out = nc.declare_dram_parameter("lv", [NP, LAT], f32, isOutput=True)

    HS, BOUNDS = cfg["HS"], cfg["BOUNDS"]
    cc_in1_p = [nc.dram_tensor(f"cc_in1_{p}", [HS[p], F1], dt1)
                for p in range(NG)]
    cc_out1_p = [nc.dram_tensor(f"cc_out1_{p}", [NCORES * HS[p], F1], dt1,
                                addr_space="Shared") for p in range(NG)]
    cc_in2_p = [nc.dram_tensor(f"cc_in2_{p}", [HS[p], F2], bf)
                for p in range(NG)]
    cc_out2_p = [nc.dram_tensor(f"cc_out2_{p}", [NCORES * HS[p], F2], bf,
                                addr_space="Shared") for p in range(NG)]

    groups = [list(range(NCORES))]

    def rows_of(t):
        return max(0, min(P, NP - t * P))

    with tile.TileContext(nc) as tc, ExitStack() as ctx:
        mm = lambda *a, **k: nc.tensor.matmul(*a, skip_group_check=True, **k)

        cp = ctx.enter_context(tc.tile_pool(name="const", bufs=1))
        iota_sb = cp.tile([P, P], bf)
        ident_sb = cp.tile([P, P], bf)
        ident8_sb = cp.tile([P, P], bf8)
        nc.sync.dma_start(out=ident8_sb[:], in_=ident8[:])
        Wsh_sb = cp.tile([P, 2 * 256], bf)
        W2_sb = cp.tile([P, 2 * F2], bf)
        deg_sb = cp.tile([P, NT], f32)
        dis_sb = cp.tile([P, NT], f32)
        nc.sync.dma_start(out=iota_sb[:], in_=iota[:])
        nc.sync.dma_start(out=ident_sb[:], in_=ident[:])
        nc.sync.dma_start(out=Wsh_sb[:], in_=Wsh[:])
        nc.sync.dma_start(out=W2_sb[:], in_=W2[:])
        nc.sync.dma_start(out=deg_sb[:], in_=degw[:])
        nc.vector.reciprocal(dis_sb[:], deg_sb[:])
        nc.scalar.activation(dis_sb[:], dis_sb[:],
                             mybir.ActivationFunctionType.Sqrt)
        brep_sb = b2rep_sb = None
        if has_b1:
            brep_sb = cp.tile([P, 256], f32)
            nc.sync.dma_start(out=brep_sb[:], in_=brep[:])
        if has_b2:
            b2rep_sb = cp.tile([P, 2 * LAT], f32)
            nc.sync.dma_start(out=b2rep_sb[:], in_=b2rep[:])

        tloc = cp.tile([P, NT * F1], dt1)    # t~ local rows (tile-major)
        m2loc = cp.tile([P, NT * F2], bf)    # m2~ local rows


        # one gpsimd register per distinct gather length (reused across calls)
        nidx_reg = {}
        for (c0, c1, _g) in calls:
            n_ = (c1 - c0) * P
            if n_ not in nidx_reg:
                nidx_reg[n_] = nc.gpsimd.to_reg(n_)

        def piece_of(r):
            for p in range(NG):
                if r < BOUNDS[p + 1]:
                    return p
            return NG - 1

        def piece_write(dests, r0, hi, sb_rows):
            """DMA sb_rows[:hi] to local rows [r0, r0+hi), split at piece
            boundaries so each piece tensor's deps are exact."""
            a = r0
            while a < r0 + hi:
                p = piece_of(a)
                b = min(r0 + hi, BOUNDS[p + 1])
                nc.sync.dma_start(out=dests[p][a - BOUNDS[p]: b - BOUNDS[p], :],
                                  in_=sb_rows[a - r0: b - r0, :])
                a = b

        # ---------------- phase 0: transform + t~ -> cc_in1 -----------------
        TB0 = 4                                  # x tiles loaded per DMA
        with tc.tile_pool(name="ph0", bufs=3) as p0, \
             tc.tile_pool(name="ph0ps", bufs=4, space="PSUM") as p0ps:
            for t0 in range(0, NT, TB0):
                ntl = min(TB0, NT - t0)
                xa = []
                for a in range(2):
                    xt4 = p0.tile([P, ntl * P], bf, tag=f"xa{a}")
                    nc.sync.dma_start(
                        out=xt4[:],
                        in_=xT[:, a * NPP + t0 * P: a * NPP + (t0 + ntl) * P])
                    xa.append(xt4)
                for ti in range(ntl):
                    t = t0 + ti
                    hi = rows_of(t)
                    if hi == 0:
                        continue
                    ps = p0ps.tile([P, F1], f32, tag="tps")
                    for a in range(2):
                        mm(ps[:], lhsT=xa[a][:, ti * P:(ti + 1) * P],
                           rhs=Wsh_sb[:, a * 256:(a + 1) * 256],
                           start=(a == 0), stop=(a == 1))
                    dst = tloc[:, t * F1:(t + 1) * F1]
                    nc.scalar.activation(dst, ps[:],
                                         mybir.ActivationFunctionType.Copy,
                                         scale=dis_sb[:, t:t + 1])
                    piece_write(cc_in1_p, t * P, hi, dst)

        def issue_ag(cc_in, cc_out):
            nc.gpsimd.collective_compute(
                "AllGather", mybir.AluOpType.bypass, replica_groups=groups,
                ins=[cc_in[:, :]], outs=[cc_out[:, :]])

        for p in range(NG):
            issue_ag(cc_in1_p[p], cc_out1_p[p])

        # ---------------- propagate (shared L1/L2) --------------------------
        batch_bounds = pre["batch_bounds"]
        calls_by_batch = [[] for _ in range(NB)]
        for (c0, c1, gg) in calls:
            for b in range(NB):
                if batch_bounds[b] <= c0 and c1 <= batch_bounds[b + 1]:
                    calls_by_batch[b].append((c0, c1, gg))
                    break
            else:
                raise AssertionError("call crosses batch bound")

        gbufs = cfg.get('GBUFS', 8)
        gp = ctx.enter_context(tc.tile_pool(name="gat", bufs=gbufs))
        mp = ctx.enter_context(tc.tile_pool(name="meta", bufs=3))
        sp = ctx.enter_context(tc.tile_pool(name="s", bufs=gbufs))
        ep = ctx.enter_context(tc.tile_pool(name="eps", bufs=4))
        qctr = [0]                      # shared gather-queue rotation

        def propagate(layer, tables, F, self_tiles, epilogue, acc_bufs, pa,
                      dt, ident_self, post_batch=None, cfg=cfg):
            psum_of = {}
            for b in range(NB):
                bc0, bc1 = batch_bounds[b], batch_bounds[b + 1]
                nbc = bc1 - bc0
                if nbc == 0:
                    continue
                # one metadata slab DMA pair per supertile batch
                idx_sl = mp.tile([P, nbc * 8], mybir.dt.int16,
                                 tag=f"idx{layer}")
                nc.sync.dma_start(out=idx_sl[:],
                                  in_=idxw[:, bc0 * 8: bc1 * 8])
                slot_sl = mp.tile([P, nbc], bf, tag=f"slot{layer}")
                nc.sync.dma_start(out=slot_sl[:], in_=slotw[:, bc0:bc1])
                for (c0, c1, gg) in calls_by_batch[b]:
                    nch_call = c1 - c0
                    nidx = nch_call * P
                    idx_t = idx_sl[:, (c0 - bc0) * 8:(c1 - bc0) * 8]
                    S_t = sp.tile([P, nch_call * P], dt, tag=f"S{layer}")
                    iota_b = bass.AP(iota_sb[:].tensor, iota_sb[:].offset,
                                     [list(iota_sb[:].ap[0]), [0, nch_call],
                                      list(iota_sb[:].ap[1])])
                    sbase = slot_sl[:, c0 - bc0: c1 - bc0]
                    slot_b = bass.AP(sbase.tensor, sbase.offset,
                                     [list(sbase.ap[0]),
                                      list(sbase.ap[1]), [0, P]])
                    nc.vector.tensor_tensor(out=S_t[:], in0=iota_b,
                                            in1=slot_b,
                                            op=mybir.AluOpType.is_equal)
                    gath = gp.tile([P, nch_call, F], dt, tag=f"g{layer}")
                    q = qctr[0] % cfg.get('NQUEUES', 1)
                    qctr[0] += 1
                    nc.gpsimd.dma_gather(
                        gath[:], tables[gg][:, :],
                        idx_t, nidx, nidx_reg[nidx], F,
                        queue_num=q,
                        single_packet=cfg.get('SINGLE_PACKET', True),
                    )
                    for ch in range(c0, c1):
                        t = int(chunk_tile[ch])
                        if ch == first_ch[t]:
                            ps = pa.tile([P, F], f32, tag=f"acc{layer}")
                            psum_of[t] = ps
                            mm(ps[:], lhsT=ident_self[:],
                               rhs=self_tiles[:, t * F:(t + 1) * F],
                               start=True, stop=False)
                        ps = psum_of[t]
                        mm(ps[:],
                           lhsT=S_t[:, (ch - c0) * P:(ch - c0 + 1) * P],
                           rhs=gath[:, ch - c0, :],
                           start=False, stop=(ch == last_ch[t]))
                        if ch == last_ch[t]:
                            epilogue(t, ps, ep)
                            del psum_of[t]
                if post_batch is not None:
                    post_batch(b)

        # ---------------- L1 epilogue: h, hhat, m2 --------------------------
        l1_ctx = ExitStack()
        pa1 = l1_ctx.enter_context(tc.tile_pool(name="acc1", bufs=BT,
                                                space="PSUM"))
        tp_pool = l1_ctx.enter_context(tc.tile_pool(name="tp", bufs=2,
                                                    space="PSUM"))
        m2_pool = l1_ctx.enter_context(tc.tile_pool(name="m2", bufs=1,
                                                    space="PSUM"))

        def epi1(t, ps, ep):
            hi = rows_of(t)
            if has_b1:
                z = ep.tile([P, F1], f32, tag="z")
                nc.scalar.activation(z[:], ps[:],
                                     mybir.ActivationFunctionType.Copy,
                                     scale=dis_sb[:, t:t + 1])
                nc.vector.tensor_tensor(out=z[:], in0=z[:], in1=brep_sb[:],
                                        op=mybir.AluOpType.add)
                h = ep.tile([P, F1], f32, tag="h")
                nc.scalar.activation(h[:], z[:],
                                     mybir.ActivationFunctionType.Lrelu,
                                     alpha=0.01)
            else:
                # h = LeakyReLU(dis * seg, 0.01) fused on the Scalar engine
                h = ep.tile([P, F1], f32, tag="h")
                nc.scalar.activation(h[:], ps[:],
                                     mybir.ActivationFunctionType.Lrelu,
                                     scale=dis_sb[:, t:t + 1], alpha=0.01)
            hhat = ep.tile([P, F1], bf, tag="hh")
            nc.scalar.activation(hhat[:], h[:],
                                 mybir.ActivationFunctionType.Copy,
                                 scale=dis_sb[:, t:t + 1])
            m2ps = m2_pool.tile([P, F2], f32, tag="m2ps")
            for a in range(2):
                tp = tp_pool.tile([P, P], bf, tag="tp")
                nc.tensor.transpose(tp[:], hhat[:, a * P:(a + 1) * P],
                                    ident_sb[:])
                hT = ep.tile([P, P], bf, tag="hT")
                nc.scalar.activation(hT[:], tp[:],
                                     mybir.ActivationFunctionType.Copy)
                mm(m2ps[:], lhsT=hT[:],
                   rhs=W2_sb[:, a * F2:(a + 1) * F2],
                   start=(a == 0), stop=(a == 1))
            dst = m2loc[:, t * F2:(t + 1) * F2]
            nc.scalar.activation(dst, m2ps[:],
                                 mybir.ActivationFunctionType.Copy)
            piece_write(cc_in2_p, t * P, hi, dst)

        # AG2 piece p fires once the last tile of piece p has its epilogue
        # issued (plus pipeline margin), overlapping the rest of prop1
        ag2_at = {}
        for p in range(NG):
            last_tile = -(-BOUNDS[p + 1] // P) - 1
            bb = min(last_tile // BT + 2, NB - 1)
            ag2_at.setdefault(bb, []).append(p)
        ag2_done = set()

        def post_batch1(b):
            for p in ag2_at.get(b, []):
                if p not in ag2_done:
                    ag2_done.add(p)
                    issue_ag(cc_in2_p[p], cc_out2_p[p])

        propagate(1, cc_out1_p, F1, tloc, epi1, acc_bufs=BT, pa=pa1,
                  dt=dt1, ident_self=ident8_sb if use_fp8 else ident_sb,
                  post_batch=post_batch1)
        for p in range(NG):
            if p not in ag2_done:
                issue_ag(cc_in2_p[p], cc_out2_p[p])
        l1_ctx.close()

        # ---------------- L2 epilogue: mu / lv ------------------------------
        def epi2(t, ps, ep):
            hi = rows_of(t)
            muv = ep.tile([P, LAT], f32, tag="mu")
            lvv = ep.tile([P, LAT], f32, tag="lv")
            if has_b2:
                o2 = ep.tile([P, 2 * LAT], f32, tag="o2")
                nc.vector.tensor_scalar_mul(o2[:], ps[:], dis_sb[:, t:t + 1])
                nc.vector.tensor_tensor(out=o2[:], in0=o2[:], in1=b2rep_sb[:],
                                        op=mybir.AluOpType.add)
                nc.vector.tensor_copy(out=muv[:], in_=o2[:, :LAT])
                nc.vector.tensor_scalar(out=lvv[:], in0=o2[:, LAT:],
                                        scalar1=10.0, scalar2=None,
                                        op0=mybir.AluOpType.min)
            else:
                nc.scalar.activation(muv[:], ps[:, :LAT],
                                     mybir.ActivationFunctionType.Copy,
                                     scale=dis_sb[:, t:t + 1])
                nc.vector.tensor_scalar(out=lvv[:], in0=ps[:, LAT:],
                                        scalar1=dis_sb[:, t:t + 1],
                                        scalar2=10.0,
                                        op0=mybir.AluOpType.mult,
                                        op1=mybir.AluOpType.min)
            nc.sync.dma_start(out=mu_out[t * P: t * P + hi, :],
                              in_=muv[:hi, :])
            nc.sync.dma_start(out=lv_out[t * P: t * P + hi, :],
                              in_=lvv[:hi, :])

        with tc.tile_pool(name="acc2", bufs=BT + 2, space="PSUM") as pa2:
            propagate(2, cc_out2_p, F2, m2loc, epi2, acc_bufs=BT + 2, pa=pa2,
                      dt=bf, ident_self=ident_sb)

    return nc


# ======================================================================
# Public entry point
# ======================================================================
def kernel(**inputs):
    """Full-input distributed GCN encoder on 8 TRN2 NeuronCores.

    Takes the unsharded inputs of reference.setup_inputs(), shards nodes
    across the 8 cores, runs the Bass kernel via run_bass_kernel_spmd,
    and returns the full (mu, logvar) tuple.
    """
    import os
    import sys
    import types

    x = np.asarray(inputs["x"], dtype=np.float32)
    edge_index = np.asarray(inputs["edge_index"])
    W_shared = np.asarray(inputs["W_shared"], dtype=np.float32)
    b_shared = np.asarray(inputs["b_shared"], dtype=np.float32)
    W_mu = np.asarray(inputs["W_mu"], dtype=np.float32)
    b_mu = np.asarray(inputs["b_mu"], dtype=np.float32)
    W_lv = np.asarray(inputs["W_lv"], dtype=np.float32)
    b_lv = np.asarray(inputs["b_lv"], dtype=np.float32)

    N = x.shape[0]
    cfg = make_cfg(
        N, ncores=8,
        call_chunks=int(os.environ.get("GCN_CALL_CHUNKS", "8")),
        nqueues=int(os.environ.get("GCN_NQUEUES", "4")),
        bt=int(os.environ.get("GCN_BT", "5")),
        dma_scratch=int(os.environ.get("GCN_DMA_SCRATCH", "32768")),
    )
    cfg['GBUFS'] = int(os.environ.get("GCN_GBUFS", "10"))
    cfg['SINGLE_PACKET'] = os.environ.get("GCN_SP", "1") == "1"
    cfg['FP8'] = os.environ.get("GCN_FP8", "0") == "1"
    cfg['AGX'] = int(os.environ.get("GCN_AGX", "1"))
    pre = preprocess(cfg, edge_index)
    in_maps, has_b1, has_b2 = stage_host(
        cfg, pre, x, W_shared, b_shared, W_mu, b_mu, W_lv, b_lv)
    nc = build_kernel(cfg, pre, has_b1, has_b2)
    nc.finalize()

    from concourse.bass_utils import run_bass_kernel_spmd

    trace = bool(int(os.environ.get("GCN_KERNEL_TRACE", "0")))
    if trace:
        # register the NTFF profiling hook this container ships without
        try:
            import trn_agent_boot.trn_boot as _tb
            _hook = _tb._ntff_profile_via_ctypes("/opt/axon/libaxon_pjrt.so")
            _m = types.ModuleType("antenv.axon_hooks")
            _m.get_axon_ntff_profile_hook = lambda: _hook
            sys.modules["antenv.axon_hooks"] = _m
        except Exception:
            trace = False

    res = run_bass_kernel_spmd(nc, in_maps, core_ids=list(range(cfg["NCORES"])),
                               trace=trace)
    kernel.last_exec_time_ns = res.exec_time_ns
    mu = np.concatenate([res.results[c]["mu"] for c in range(cfg["NCORES"])])
    lv = np.concatenate([res.results[c]["lv"] for c in range(cfg["NCORES"])])
    return mu.astype(np.float32), lv.astype(np.float32)


kernel.last_exec_time_ns = None

